# revision 21
# baseline (speedup 1.0000x reference)
"""Trainium2 Bass kernel for nn_MultiHeadContrastive (two-head contrastive loss).

Strategy (8 NeuronCores, two SPMD launches, no collectives):

  Launch 1 (MLP): rows of roi_feats are sorted by group
  (anchor / fg-low-iou / bg / ignore) on the host and sharded contiguously,
  1024 rows per core.  Both layers run in fp8e4(e4m3) with DoubleRow perf
  mode (weights pre-scaled by 64 on the host so fp8 stays in its normal
  range; the scale cancels in the host-side normalization).  Raw
  (pre-normalization, pre-b2) embeddings return as bf16; the host adds b2
  and L2-normalizes in float64.

  Launch 2 (SIM): each core owns nblk*128 anchor rows (lhsT, fp8 x16) and
  all N keys (rhs, fp8-DR).  Keys are RE-ORDERED per core so the core's own
  anchors come first: every anchor's self-similarity column then lands in
  group 0, which is always evaluated by ACT (exact exp), so the host can
  subtract exact self terms.  Per anchor block, sim rows are built in PSUM
  2048 keys at a time (2 regions, double buffered).  exp(sim/TAU) plus the
  masked range sums are computed by THREE engines concurrently:
    - ACT: exact exp + accum_out on whole groups (plus one extension piece)
    - Pool: Schraudolph exp approximation (affine in the exponent, stored
      int16, re-read as bf16) on its column span of the remaining groups
    - DVE: same Schraudolph on its span, plus ALL range sums of the int16
      tiles via tensor_scalar+accum_out (4x DVE perf mode).
  Because rows are sorted, all masked sums are plain column-range sums.

  Host: subtracts exact self terms, computes the class-positive term of the
  SupCon loss from per-class sums of z, applies logs/weights in float64.
"""

import math
import os

import numpy as np
import ml_dtypes

import concourse.bacc as bacc
import concourse.mybir as mybir
import concourse.tile as tile
from concourse.bass_utils import run_bass_kernel_spmd

N_CORES = 8
N, C = 8192, 1024
HID, DF, DC = 256, 64, 128
HID2 = 2 * HID
TAU = 0.2
EPS = 1e-8
EPS12 = 1e-12
IOU_THRESHOLD = 0.5

F32 = mybir.dt.float32
BF16 = mybir.dt.bfloat16
F8 = mybir.dt.float8e4
I16 = mybir.dt.int16
ACT = mybir.ActivationFunctionType
AX = mybir.AxisListType
ALU = mybir.AluOpType
DR = mybir.MatmulPerfMode.DoubleRow

NP8 = ml_dtypes.float8_e4m3
NPBF = ml_dtypes.bfloat16

W1SCALE = 64.0
ZSCALE = 16.0
SCH_SIG = 0.05394  # zero-mean rel. error under float->int16 truncation

LAST_RESULTS = []
LAST_TIMES = []
_NC_CACHE = {}

G = 2048
# per-group column shares (out of 2048): ACT [0,A_SH), Pool [A_SH,A_SH+P_SH),
# DVE-own the rest; short groups prorate.
A_SH = 846
P_SH = 716


def _schraudolph_consts(psum_scale):
    c = 128.0 * psum_scale * np.log2(np.e)
    b = 128.0 * (127.0 - SCH_SIG)
    return float(c), float(b)


# --------------------------------------------------------------------------
# SIM piece table (shared between device builder and host reassembly)
# --------------------------------------------------------------------------
def _sim_piece_table(n_fg, n_valid, act_ext):
    """Returns (pieces, order).

    pieces: list of dicts: head ('f'|'c'), group g, span [c0,c1) local to the
    group, engine in {'A','P','D'}, plus 'splits': sorted class-boundary cuts
    within the span (absolute head-column space).  Each (engine-span x split
    interval) becomes one stat column, assigned in order.

    order: group emission order [(head, g), ...] interleaving ACT-owned and
    Pool/DVE-owned groups for engine overlap.
    """
    NGH = N // G                  # 4 groups per head
    pieces = []

    def add(head, g, c0, c1, eng, cuts=()):
        lo = g * G
        cs = sorted({c0 + lo, c1 + lo} | {c for c in cuts if c0 + lo < c < c1 + lo})
        for a, b in zip(cs[:-1], cs[1:]):
            pieces.append(dict(head=head, g=g, c0=a - lo, c1=b - lo, eng=eng))

    def grp(head, g, L, cuts=()):
        # every group gets all three engines: ACT / Pool / DVE column spans
        a = int(round(A_SH * L / G)) // 2 * 2
        p = int(round(P_SH * L / G)) // 2 * 2
        add(head, g, 0, a, "A", cuts)
        add(head, g, a, a + p, "P", cuts)
        if a + p < L:
            add(head, g, a + p, L, "D", cuts)

    lim = {"f": N, "c": n_valid}
    order = []
    for g in range(NGH):
        for head in ("f", "c"):
            L = min(G, lim[head] - g * G)
            if L <= 0:
                continue
            order.append((head, g))
            grp(head, g, L, cuts=(n_fg,) if head == "f" else ())
    # stat numbering: ACT pieces first (their own stat tile), then Pool/DVE
    na = 0
    for p in pieces:
        if p["eng"] == "A":
            p["stat"] = na
            na += 1
    nd = 0
    for p in pieces:
        if p["eng"] != "A":
            p["stat"] = na + nd
            nd += 1
    return pieces, order, na, nd


# --------------------------------------------------------------------------
# Launch 1: MLP (both heads, fp8 DoubleRow)
# --------------------------------------------------------------------------
def _build_mlp_nc(layer2_fp8):
    R = N // N_CORES
    RB = 512
    NR = R // RB
    KC4 = C // 256
    NH4 = HID2 // 128

    nc = bacc.Bacc(trn_type="TRN2", num_devices=N_CORES, debug=False)
    x_d = nc.dram_tensor("x", [128, KC4, 2, R], F8, kind="ExternalInput")
    w1_d = nc.dram_tensor("w1", [128, KC4, 2, HID2], F8, kind="ExternalInput")
    w2dt = F8 if layer2_fp8 else BF16
    w2_d = nc.dram_tensor("w2", [128, 2, DF + DC], w2dt, kind="ExternalInput")
    b1_d = nc.dram_tensor("b1", [128, NH4], F32, kind="ExternalInput")
    zf_d = nc.dram_tensor("zf", [DF, R], BF16, kind="ExternalOutput")
    zc_d = nc.dram_tensor("zc", [DC, R], BF16, kind="ExternalOutput")

    hdt = F8 if layer2_fp8 else BF16
    with tile.TileContext(nc) as tc:
        with (
            tc.tile_pool(name="cst", bufs=1) as cst,
            tc.tile_pool(name="hb", bufs=2) as hb,
            tc.tile_pool(name="zb", bufs=2) as zb,
            tc.tile_pool(name="ps", bufs=1, space="PSUM") as ps,
        ):
            wu = cst.tile([1, 8], F32, tag="wu")
            nc.vector.memset(wu[:, :], 0.0)
            nc.scalar.activation(out=wu[:, :], in_=wu[:, :], func=ACT.Relu, scale=1.0)

            # DMA order: w1(k01), x0(k01), w1(k23), x0(k23), x1, b1, w2
            # separate tiles per DMA so RAW deps stay piece-accurate
            w1_t = [cst.tile([128, 2, 2, HID2], F8, tag=f"w1{i}", name=f"w1{i}") for i in range(2)]
            x_t = {}
            for r in range(NR):
                for i in range(2):
                    x_t[(r, i)] = cst.tile([128, 2, 2, RB], F8, tag=f"x{r}{i}", name=f"x{r}{i}")
            nc.sync.dma_start(out=w1_t[0][:, :, :, :], in_=w1_d[:, 0:2, :, :])
            nc.sync.dma_start(out=x_t[(0, 0)][:, :, :, :], in_=x_d[:, 0:2, :, 0:RB])
            nc.sync.dma_start(out=w1_t[1][:, :, :, :], in_=w1_d[:, 2:4, :, :])
            nc.sync.dma_start(out=x_t[(0, 1)][:, :, :, :], in_=x_d[:, 2:4, :, 0:RB])
            nc.sync.dma_start(out=x_t[(1, 0)][:, :, :, :], in_=x_d[:, 0:2, :, RB:R])
            nc.sync.dma_start(out=x_t[(1, 1)][:, :, :, :], in_=x_d[:, 2:4, :, RB:R])
            b1_t = cst.tile([128, NH4], F32, tag="b1")
            nc.sync.dma_start(out=b1_t[:, :], in_=b1_d[:, :])
            w2_t = cst.tile([128, 2, DF + DC], w2dt, tag="w2")
            nc.sync.dma_start(out=w2_t[:, :, :], in_=w2_d[:, :, :])

            # 8 psum banks: hp[r][h4] for both rblocks; z reuses drained banks
            hp = {(r, h4): ps.tile([128, RB], F32, tag=f"hp{r}{h4}",
                                   name=f"hp{r}{h4}")
                  for r in range(NR) for h4 in range(NH4)}
            for r in range(NR):
                # engine-pure hidden tiles: DVE writes head f, ACT head c
                hsb = [hb.tile([128, 2, RB], hdt, tag=f"h{r}f", name=f"h{r}f"),
                       hb.tile([128, 2, RB], hdt, tag=f"h{r}c", name=f"h{r}c")]
                for h4 in range(NH4):
                    for k in range(KC4):
                        nc.tensor.matmul(
                            out=hp[(r, h4)][:, :],
                            lhsT=w1_t[k // 2][:, k % 2, :, h4 * 128:(h4 + 1) * 128],
                            rhs=x_t[(r, k // 2)][:, k % 2, :, :],
                            start=(k == 0),
                            stop=(k == KC4 - 1),
                            perf_mode=DR,
                        )
                    if h4 < 2:
                        nc.vector.tensor_scalar(
                            out=hsb[0][:, h4, :], in0=hp[(r, h4)][:, :],
                            scalar1=b1_t[:, h4:h4 + 1], scalar2=0.0,
                            op0=ALU.add, op1=ALU.max)
                    else:
                        nc.scalar.activation(
                            out=hsb[1][:, h4 - 2, :], in_=hp[(r, h4)][:, :],
                            func=ACT.Relu, bias=b1_t[:, h4:h4 + 1], scale=1.0)
                for hi, (d, zd) in enumerate(((DF, zf_d), (DC, zc_d))):
                    c0 = 0 if hi == 0 else DF
                    zp = ps.tile([d, RB], F32, tag=f"hp{r}{hi}", name=f"zp{r}{hi}")
                    if layer2_fp8:
                        nc.tensor.matmul(
                            out=zp[:, :], lhsT=w2_t[:, :, c0:c0 + d],
                            rhs=hsb[hi][:, :, :],
                            start=True, stop=True, perf_mode=DR)
                    else:
                        for kk in range(2):
                            nc.tensor.matmul(
                                out=zp[:, :], lhsT=w2_t[:, kk, c0:c0 + d],
                                rhs=hsb[hi][:, kk, :],
                                start=(kk == 0), stop=(kk == 1))
                    zt = zb.tile([d, RB], BF16, tag=f"z{hi}")
                    if hi == 0:
                        nc.vector.tensor_scalar(
                            out=zt[:, :], in0=zp[:, :], scalar1=1.0,
                            scalar2=None, op0=ALU.mult)
                    else:
                        nc.scalar.activation(
                            out=zt[:, :], in_=zp[:, :], func=ACT.Copy,
                            bias=0.0, scale=1.0)
                    nc.sync.dma_start(out=zd[:, r * RB:(r + 1) * RB], in_=zt[:, :])
    nc.compile()
    return nc


# --------------------------------------------------------------------------
# Launch 2: SIM
# --------------------------------------------------------------------------
def _build_sim_nc(n_fg, n_valid, nblk, act_ext):
    A = nblk * 128
    pieces, order, NA, ND = _sim_piece_table(n_fg, n_valid, act_ext)
    act_scale = 1.0 / (ZSCALE * ZSCALE * TAU)
    sch_c, sch_b = _schraudolph_consts(act_scale)

    nc = bacc.Bacc(trn_type="TRN2", num_devices=N_CORES, debug=False)
    zfk_d = nc.dram_tensor("zfk", [32, 2, N], F8, kind="ExternalInput")
    zck_d = nc.dram_tensor("zck", [64, 2, N], F8, kind="ExternalInput")
    zfa_d = nc.dram_tensor("zfa", [32, 2, A], F8, kind="ExternalInput")
    zca_d = nc.dram_tensor("zca", [64, 2, A], F8, kind="ExternalInput")
    sta_d = nc.dram_tensor("sta", [nblk, 128, NA], F32, kind="ExternalOutput")
    std_d = nc.dram_tensor("std", [nblk, 128, ND], F32, kind="ExternalOutput")

    by_group = {}
    for p in pieces:
        by_group.setdefault((p["head"], p["g"]), []).append(p)

    H = N // 2  # keys per half-tile (separate tiles so group RAW deps are tight)
    with tile.TileContext(nc) as tc:
        with (
            tc.tile_pool(name="keys", bufs=1) as keys,
            tc.tile_pool(name="anch", bufs=1) as anch,
            tc.tile_pool(name="sch", bufs=2) as sch,
            tc.tile_pool(name="st", bufs=2) as st,
            tc.tile_pool(name="ps", bufs=1, space="PSUM") as ps,
        ):
            zfa_t = anch.tile([32, 2, A], F8, tag="zfa")
            nc.sync.dma_start(out=zfa_t[:, :, :], in_=zfa_d[:, :, :])
            wu = st.tile([1, 8], F32, tag="wu")
            nc.vector.memset(wu[:, :], 0.0)
            nc.scalar.activation(out=wu[:, :], in_=wu[:, :], func=ACT.Exp, scale=1.0)
            zfk_t, zck_t = [], []
            zfk_t.append(keys.tile([32, 2, H], F8, tag="zfk0", name="zfk0"))
            nc.sync.dma_start(out=zfk_t[0][:, :, :], in_=zfk_d[:, :, 0:H])
            zca_t = anch.tile([64, 2, A], F8, tag="zca")
            nc.sync.dma_start(out=zca_t[:, :, :], in_=zca_d[:, :, :])
            zck_t.append(keys.tile([64, 2, H], F8, tag="zck0", name="zck0"))
            nc.sync.dma_start(out=zck_t[0][:, :, :], in_=zck_d[:, :, 0:H])
            zfk_t.append(keys.tile([32, 2, H], F8, tag="zfk1", name="zfk1"))
            nc.sync.dma_start(out=zfk_t[1][:, :, :], in_=zfk_d[:, :, H:N])
            zck_t.append(keys.tile([64, 2, H], F8, tag="zck1", name="zck1"))
            nc.sync.dma_start(out=zck_t[1][:, :, :], in_=zck_d[:, :, H:N])

            pr = [ps.tile([128, G], F32, tag=f"pr{i}", name=f"pr{i}")
                  for i in range(2)]

            for ab in range(nblk):
                stA = st.tile([128, NA], F32, tag="stA")
                stD = st.tile([128, ND], F32, tag="stD")
                dummy = st.tile([128, G], BF16, tag="dummy")
                for oi, (head, g) in enumerate(order):
                    region = pr[oi % 2]
                    lhsT = (zfa_t if head == "f" else zca_t)[:, :, ab * 128:(ab + 1) * 128]
                    kt = (zfk_t if head == "f" else zck_t)[(g * G) // H]
                    koff = g * G - ((g * G) // H) * H
                    gp = by_group[(head, g)]
                    kmax = max(p["c1"] for p in gp)
                    nmm = int(math.ceil(kmax / 512))

                    def mm(kk):
                        nc.tensor.matmul(
                            out=region[:, kk * 512:(kk + 1) * 512],
                            lhsT=lhsT,
                            rhs=kt[:, :, koff + kk * 512:koff + (kk + 1) * 512],
                            start=True, stop=True, perf_mode=DR)

                    for kk in range(min(2, nmm)):
                        mm(kk)
                    for p in gp:
                        if p["eng"] == "A":
                            assert p["c1"] <= 1024
                            nc.scalar.activation(
                                out=region[:, p["c0"]:p["c1"]],
                                in_=region[:, p["c0"]:p["c1"]],
                                func=ACT.Exp, scale=act_scale,
                                accum_out=stA[:, p["stat"]:p["stat"] + 1])
                    for kk in range(2, nmm):
                        mm(kk)
                    i16 = {}
                    for eng, engine in (("P", nc.gpsimd), ("D", nc.vector)):
                        sp = [p for p in gp if p["eng"] == eng]
                        if not sp:
                            continue
                        c0 = min(p["c0"] for p in sp)
                        c1 = max(p["c1"] for p in sp)
                        t16 = sch.tile([128, c1 - c0], I16, tag=f"i16{eng}{oi}")
                        i16[eng] = (t16, c0)
                        engine.tensor_scalar(
                            out=t16[:, :], in0=region[:, c0:c1],
                            scalar1=sch_c, scalar2=sch_b,
                            op0=ALU.mult, op1=ALU.add)
                    for p in gp:
                        if p["eng"] in "PD":
                            t16, c0 = i16[p["eng"]]
                            nc.vector.tensor_scalar(
                                out=dummy[:, p["c0"]:p["c1"]],
                                in0=t16[:, p["c0"] - c0:p["c1"] - c0].bitcast(BF16),
                                scalar1=1.0, scalar2=0.0,
                                op0=ALU.mult, op1=ALU.add,
                                accum_out=stD[:, p["stat"] - NA:p["stat"] - NA + 1])
                nc.sync.dma_start(out=sta_d[ab, :, :], in_=stA[:, :])
                nc.sync.dma_start(out=std_d[ab, :, :], in_=stD[:, :])
    nc.compile()
    return nc


def _run(nc, in_maps, out_names):
    import time as _time

    if os.environ.get("CC_BASS_SIM") == "1":
        from concourse import bass_interp

        ncores = int(os.environ.get("CC_BASS_SIM_CORES", str(N_CORES)))
        results = []
        for m in range(ncores):
            sim = bass_interp.CoreSim(nc, core_id=m)
            for k, v in in_maps[m].items():
                sim.tensor(k)[:] = v
            if nc.partition_id_tensor is not None:
                sim.tensor(nc.partition_id_tensor.name)[:] = np.array(
                    [[m]], dtype=np.uint32)
            sim.simulate()
            results.append(
                {name: np.array(sim.mem_tensor(name)) for name in out_names})
        while len(results) < N_CORES:
            results.append(results[-1])
        return results
    t0 = _time.monotonic()
    res = run_bass_kernel_spmd(nc, in_maps, core_ids=list(range(N_CORES)))
    LAST_TIMES.append(_time.monotonic() - t0)
    LAST_RESULTS.append(res)
    return res.results


def kernel(**inputs):
    global LAST_RESULTS, LAST_TIMES
    LAST_RESULTS = []
    LAST_TIMES = []

    roi = np.ascontiguousarray(np.asarray(inputs["roi_feats"], dtype=np.float32))
    labels = np.asarray(inputs["labels"]).astype(np.int64)
    ious = np.asarray(inputs["ious"], dtype=np.float32)
    w1f = np.asarray(inputs["w1f"], dtype=np.float64)
    b1f = np.asarray(inputs["b1f"], dtype=np.float64)
    w2f = np.asarray(inputs["w2f"], dtype=np.float64)
    b2f = np.asarray(inputs["b2f"], dtype=np.float64)
    w1c = np.asarray(inputs["w1c"], dtype=np.float64)
    b1c = np.asarray(inputs["b1c"], dtype=np.float64)
    w2c = np.asarray(inputs["w2c"], dtype=np.float64)
    b2c = np.asarray(inputs["b2c"], dtype=np.float64)
    assert roi.shape == (N, C)

    ign = labels == -1
    fg = (labels > 0) & ~ign
    bg = (labels == 0) & ~ign
    anc = fg & (ious > IOU_THRESHOLD)

    perm = np.concatenate(
        [np.where(anc)[0], np.where(fg & ~anc)[0], np.where(bg)[0], np.where(ign)[0]])
    n_A = int(anc.sum())
    n_fg = int(fg.sum())
    n_valid = n_fg + int(bg.sum())
    if n_A == 0:
        return np.zeros(2, dtype=np.float32)

    x_s = roi[perm]
    labels_s = labels[perm]
    ious_s = ious[perm].astype(np.float64)

    # ---------------- launch 1: MLP ----------------
    w1_all = np.concatenate([w1f, w1c], axis=0)
    b1_all = np.concatenate([b1f, b1c], axis=0) * W1SCALE
    w1_q = (w1_all * W1SCALE).astype(NP8)
    x_q = x_s.astype(NP8)

    h_probe = np.maximum(
        x_q[:256].astype(np.float32) @ w1_q.astype(np.float32).T
        + b1_all.astype(np.float32), 0)
    layer2_fp8 = bool(h_probe.max() < 200.0)

    mlp_key = ("mlp", layer2_fp8)
    if mlp_key not in _NC_CACHE:
        _NC_CACHE[mlp_key] = _build_mlp_nc(layer2_fp8)
    nc1 = _NC_CACHE[mlp_key]

    KC4 = C // 256
    R = N // N_CORES
    w1_dr = np.ascontiguousarray(
        w1_q.T.reshape(KC4, 2, 128, HID2).transpose(2, 0, 1, 3))
    w2_all = np.concatenate([w2f, w2c], axis=0)
    w2dt = NP8 if layer2_fp8 else NPBF
    w2_dr = np.ascontiguousarray(
        w2_all.T.reshape(2, 128, DF + DC).transpose(1, 0, 2)).astype(w2dt)
    b1_dr = np.ascontiguousarray(
        b1_all.reshape(HID2 // 128, 128).T).astype(np.float32)

    xT_q = np.ascontiguousarray(x_q.T)
    shared1 = {"w1": w1_dr, "w2": w2_dr, "b1": b1_dr}
    in_maps1 = []
    for m in range(N_CORES):
        xm = xT_q[:, m * R:(m + 1) * R]
        x_dr = np.ascontiguousarray(
            xm.reshape(KC4, 2, 128, R).transpose(2, 0, 1, 3))
        in_maps1.append({"x": x_dr, **shared1})
    res1 = _run(nc1, in_maps1, ["zf", "zc"])

    zfT_raw = np.concatenate([r["zf"].astype(np.float64) for r in res1], axis=1)
    zcT_raw = np.concatenate([r["zc"].astype(np.float64) for r in res1], axis=1)

    def _normalize(zT_raw, b2):
        z = zT_raw.T + b2[None, :] * W1SCALE
        nrm = np.sqrt(np.sum(z * z, axis=1, keepdims=True)) / W1SCALE
        return z / W1SCALE / np.maximum(nrm, EPS)

    zfn = _normalize(zfT_raw, b2f)
    zcn = _normalize(zcT_raw, b2c)

    zfq = (zfn * ZSCALE).astype(NP8)
    zcq = (zcn * ZSCALE).astype(NP8)
    zfq64 = zfq.astype(np.float64)
    zcq64 = zcq.astype(np.float64)

    # ---------------- launch 2: sims ----------------
    nblk = max(1, math.ceil(math.ceil(n_A / N_CORES) / 128))
    A_pc = nblk * 128
    # ACT extension into cls G1 to balance engines (rounded to 16)
    act_ext = 966 // 16 * 16
    sim_key = ("sim", n_fg, n_valid, nblk, act_ext)
    if sim_key not in _NC_CACHE:
        _NC_CACHE[sim_key] = _build_sim_nc(n_fg, n_valid, nblk, act_ext)
    nc2 = _NC_CACHE[sim_key]
    pieces, _, NA, ND = _sim_piece_table(n_fg, n_valid, act_ext)

    def _dr(zq_cols):
        d = zq_cols.shape[0]
        return np.ascontiguousarray(zq_cols.reshape(2, d // 2, -1).transpose(1, 0, 2))

    zfqT = np.ascontiguousarray(zfq.T)   # [DF, N]
    zcqT = np.ascontiguousarray(zcq.T)   # [DC, N]
    in_maps2 = []
    for m in range(N_CORES):
        lo = min(m * A_pc, n_A)
        hi = min((m + 1) * A_pc, n_A)
        # local key order: own anchor window first
        local = np.concatenate([
            np.arange(lo, hi),
            np.arange(0, lo),
            np.arange(hi, N),
        ])
        aidx = np.minimum(np.arange(m * A_pc, (m + 1) * A_pc), n_A - 1)
        in_maps2.append({
            "zfk": _dr(zfqT[:, local]),
            "zck": _dr(zcqT[:, local]),
            "zfa": _dr(zfqT[:, aidx]),
            "zca": _dr(zcqT[:, aidx]),
        })
    res2 = _run(nc2, in_maps2, ["sta", "std"])

    NSTAT = len(pieces)
    stats = np.stack(
        [np.concatenate([r["sta"].reshape(A_pc, NA),
                         r["std"].reshape(A_pc, ND)], axis=1) for r in res2],
        axis=0).astype(np.float64)        # [cores, A_pc, NSTAT]

    # ---------------- host: final losses in float64 ----------------
    act_scale = 1.0 / (ZSCALE * ZSCALE * TAU)
    # piece -> class membership (per-core local column space; class sections
    # are preserved by the local reordering, so boundaries are global)
    numer_cols = [p["stat"] for p in pieces
                  if p["head"] == "f" and p["g"] * G + p["c1"] <= n_fg]
    denom_cols = [p["stat"] for p in pieces if p["head"] == "f"]
    denc_cols = [p["stat"] for p in pieces if p["head"] == "c"]

    out_rows = np.empty((n_A, NSTAT), dtype=np.float64)
    for m in range(N_CORES):
        lo = m * A_pc
        hi = min((m + 1) * A_pc, n_A)
        if hi > lo:
            out_rows[lo:hi] = stats[m, : hi - lo]
    stats = out_rows                      # [n_A, NSTAT]

    w_a = ious_s[:n_A]
    sdot_f = np.einsum("nd,nd->n", zfq64[:n_A], zfq64[:n_A])
    sdot_c = np.einsum("nd,nd->n", zcq64[:n_A], zcq64[:n_A])
    selfexp_f = np.exp(sdot_f * act_scale)
    selfexp_c = np.exp(sdot_c * act_scale)

    numer = stats[:, numer_cols].sum(1) - selfexp_f
    denom = stats[:, denom_cols].sum(1) - selfexp_f
    denom_c = stats[:, denc_cols].sum(1) - selfexp_c

    if n_fg - 1 > 0:
        li = -np.log((numer + EPS) / (denom + EPS))
        loss_fg = np.sum(li * w_a) / (np.sum(w_a) + EPS)
    else:
        loss_fg = 0.0

    lab_valid = labels_s[:n_valid]
    cnt = np.bincount(lab_valid, minlength=21)
    S = np.zeros((21, DC), dtype=np.float64)
    np.add.at(S, lab_valid, zcn[:n_valid])
    c_a = labels_s[:n_A]
    n_pos = (cnt[c_a] - 1).astype(np.float64)
    denom_log = np.log(np.maximum(denom_c, 1e-300))
    zca64 = zcn[:n_A]
    sum_pos = (np.einsum("nd,nd->n", zca64, S[c_a])
               - np.einsum("nd,nd->n", zca64, zca64)) / TAU
    li_c = -(sum_pos - n_pos * denom_log) / np.maximum(n_pos, 1.0)
    valid_c = n_pos > 0
    num2 = np.sum(np.where(valid_c, li_c * w_a, 0.0))
    den2 = np.sum(np.where(valid_c, w_a, 0.0))
    loss_cls = num2 / (den2 + EPS12)

    return np.stack([loss_fg, loss_cls]).astype(np.float32)


# revision 22
# speedup vs baseline: 1.0033x; 1.0033x over previous
"""Trainium2 Bass kernel for nn_MultiHeadContrastive (two-head contrastive loss).

Strategy (8 NeuronCores, two SPMD launches, no collectives):

  Launch 1 (MLP): rows of roi_feats are sorted by group
  (anchor / fg-low-iou / bg / ignore) on the host and sharded contiguously,
  1024 rows per core.  Both layers run in fp8e4(e4m3) with DoubleRow perf
  mode (weights pre-scaled by 64 on the host so fp8 stays in its normal
  range; the scale cancels in the host-side normalization).  Raw
  (pre-normalization, pre-b2) embeddings return as bf16; the host adds b2
  and L2-normalizes in float64.

  Launch 2 (SIM): each core owns nblk*128 anchor rows (lhsT, fp8 x16) and
  all N keys (rhs, fp8-DR).  Keys are RE-ORDERED per core so the core's own
  anchors come first: every anchor's self-similarity column then lands in
  group 0, which is always evaluated by ACT (exact exp), so the host can
  subtract exact self terms.  Per anchor block, sim rows are built in PSUM
  2048 keys at a time (2 regions, double buffered).  exp(sim/TAU) plus the
  masked range sums are computed by THREE engines concurrently:
    - ACT: exact exp + accum_out on whole groups (plus one extension piece)
    - Pool: Schraudolph exp approximation (affine in the exponent, stored
      int16, re-read as bf16) on its column span of the remaining groups
    - DVE: same Schraudolph on its span, plus ALL range sums of the int16
      tiles via tensor_scalar+accum_out (4x DVE perf mode).
  Because rows are sorted, all masked sums are plain column-range sums.

  Host: subtracts exact self terms, computes the class-positive term of the
  SupCon loss from per-class sums of z, applies logs/weights in float64.
"""

import math
import os

import numpy as np
import ml_dtypes

import concourse.bacc as bacc
import concourse.mybir as mybir
import concourse.tile as tile
from concourse.bass_utils import run_bass_kernel_spmd

N_CORES = 8
N, C = 8192, 1024
HID, DF, DC = 256, 64, 128
HID2 = 2 * HID
TAU = 0.2
EPS = 1e-8
EPS12 = 1e-12
IOU_THRESHOLD = 0.5

F32 = mybir.dt.float32
BF16 = mybir.dt.bfloat16
F8 = mybir.dt.float8e4
I16 = mybir.dt.int16
ACT = mybir.ActivationFunctionType
AX = mybir.AxisListType
ALU = mybir.AluOpType
DR = mybir.MatmulPerfMode.DoubleRow

NP8 = ml_dtypes.float8_e4m3
NPBF = ml_dtypes.bfloat16

W1SCALE = 64.0
ZSCALE = 16.0
SCH_SIG = 0.05394  # zero-mean rel. error under float->int16 truncation

LAST_RESULTS = []
LAST_TIMES = []
_NC_CACHE = {}

G = 2048
# per-group column shares (out of 2048): ACT [0,A_SH), Pool [A_SH,A_SH+P_SH),
# DVE-own the rest; short groups prorate.
A_SH = 846
P_SH = 716


def _schraudolph_consts(psum_scale):
    c = 128.0 * psum_scale * np.log2(np.e)
    b = 128.0 * (127.0 - SCH_SIG)
    return float(c), float(b)


# --------------------------------------------------------------------------
# SIM piece table (shared between device builder and host reassembly)
# --------------------------------------------------------------------------
def _sim_piece_table(n_fg, n_valid, act_ext):
    """Returns (pieces, order).

    pieces: list of dicts: head ('f'|'c'), group g, span [c0,c1) local to the
    group, engine in {'A','P','D'}, plus 'splits': sorted class-boundary cuts
    within the span (absolute head-column space).  Each (engine-span x split
    interval) becomes one stat column, assigned in order.

    order: group emission order [(head, g), ...] interleaving ACT-owned and
    Pool/DVE-owned groups for engine overlap.
    """
    NGH = N // G                  # 4 groups per head
    pieces = []

    def add(head, g, c0, c1, eng, cuts=()):
        lo = g * G
        cs = sorted({c0 + lo, c1 + lo} | {c for c in cuts if c0 + lo < c < c1 + lo})
        for a, b in zip(cs[:-1], cs[1:]):
            pieces.append(dict(head=head, g=g, c0=a - lo, c1=b - lo, eng=eng))

    def grp(head, g, L, cuts=()):
        # every group gets all three engines: ACT / Pool / DVE column spans
        a = int(round(A_SH * L / G)) // 2 * 2
        p = int(round(P_SH * L / G)) // 2 * 2
        add(head, g, 0, a, "A", cuts)
        add(head, g, a, a + p, "P", cuts)
        if a + p < L:
            add(head, g, a + p, L, "D", cuts)

    lim = {"f": N, "c": n_valid}
    order = []
    for g in range(NGH):
        for head in ("f", "c"):
            L = min(G, lim[head] - g * G)
            if L <= 0:
                continue
            order.append((head, g))
            grp(head, g, L, cuts=(n_fg,) if head == "f" else ())
    # stat numbering: ACT pieces first (their own stat tile), then Pool/DVE
    na = 0
    for p in pieces:
        if p["eng"] == "A":
            p["stat"] = na
            na += 1
    nd = 0
    for p in pieces:
        if p["eng"] != "A":
            p["stat"] = na + nd
            nd += 1
    return pieces, order, na, nd


# --------------------------------------------------------------------------
# Launch 1: MLP (both heads, fp8 DoubleRow)
# --------------------------------------------------------------------------
def _build_mlp_nc(layer2_fp8):
    R = N // N_CORES
    RB = 512
    NR = R // RB
    KC4 = C // 256
    NH4 = HID2 // 128

    nc = bacc.Bacc(trn_type="TRN2", num_devices=N_CORES, debug=False)
    x_d = nc.dram_tensor("x", [128, KC4, 2, R], F8, kind="ExternalInput")
    w1_d = nc.dram_tensor("w1", [128, KC4, 2, HID2], F8, kind="ExternalInput")
    w2dt = F8 if layer2_fp8 else BF16
    w2_d = nc.dram_tensor("w2", [128, 2, DF + DC], w2dt, kind="ExternalInput")
    b1_d = nc.dram_tensor("b1", [128, NH4], F32, kind="ExternalInput")
    zf_d = nc.dram_tensor("zf", [DF, R], BF16, kind="ExternalOutput")
    zc_d = nc.dram_tensor("zc", [DC, R], BF16, kind="ExternalOutput")

    hdt = F8 if layer2_fp8 else BF16
    with tile.TileContext(nc) as tc:
        with (
            tc.tile_pool(name="cst", bufs=1) as cst,
            tc.tile_pool(name="hb", bufs=2) as hb,
            tc.tile_pool(name="zb", bufs=2) as zb,
            tc.tile_pool(name="ps", bufs=1, space="PSUM") as ps,
        ):
            wu = cst.tile([1, 8], F32, tag="wu")
            nc.vector.memset(wu[:, :], 0.0)
            nc.scalar.activation(out=wu[:, :], in_=wu[:, :], func=ACT.Relu, scale=1.0)

            # DMA order: w1(k01), x0(k01), w1(k23), x0(k23), x1, b1, w2
            # separate tiles per DMA so RAW deps stay piece-accurate
            w1_t = [cst.tile([128, 2, 2, HID2], F8, tag=f"w1{i}", name=f"w1{i}") for i in range(2)]
            x_t = {}
            for r in range(NR):
                for i in range(2):
                    x_t[(r, i)] = cst.tile([128, 2, 2, RB], F8, tag=f"x{r}{i}", name=f"x{r}{i}")
            nc.sync.dma_start(out=w1_t[0][:, :, :, :], in_=w1_d[:, 0:2, :, :])
            nc.sync.dma_start(out=x_t[(0, 0)][:, :, :, :], in_=x_d[:, 0:2, :, 0:RB])
            nc.sync.dma_start(out=w1_t[1][:, :, :, :], in_=w1_d[:, 2:4, :, :])
            nc.sync.dma_start(out=x_t[(0, 1)][:, :, :, :], in_=x_d[:, 2:4, :, 0:RB])
            nc.sync.dma_start(out=x_t[(1, 0)][:, :, :, :], in_=x_d[:, 0:2, :, RB:R])
            nc.sync.dma_start(out=x_t[(1, 1)][:, :, :, :], in_=x_d[:, 2:4, :, RB:R])
            b1_t = cst.tile([128, NH4], F32, tag="b1")
            nc.sync.dma_start(out=b1_t[:, :], in_=b1_d[:, :])
            w2_t = cst.tile([128, 2, DF + DC], w2dt, tag="w2")
            nc.sync.dma_start(out=w2_t[:, :, :], in_=w2_d[:, :, :])

            # 8 psum banks: hp[r][h4] for both rblocks; z reuses drained banks
            hp = {(r, h4): ps.tile([128, RB], F32, tag=f"hp{r}{h4}",
                                   name=f"hp{r}{h4}")
                  for r in range(NR) for h4 in range(NH4)}
            for r in range(NR):
                # engine-pure hidden tiles: DVE writes head f, ACT head c
                hsb = [hb.tile([128, 2, RB], hdt, tag=f"h{r}f", name=f"h{r}f"),
                       hb.tile([128, 2, RB], hdt, tag=f"h{r}c", name=f"h{r}c")]
                for h4 in range(NH4):
                    for k in range(KC4):
                        nc.tensor.matmul(
                            out=hp[(r, h4)][:, :],
                            lhsT=w1_t[k // 2][:, k % 2, :, h4 * 128:(h4 + 1) * 128],
                            rhs=x_t[(r, k // 2)][:, k % 2, :, :],
                            start=(k == 0),
                            stop=(k == KC4 - 1),
                            perf_mode=DR,
                        )
                    if h4 < 2:
                        nc.vector.tensor_scalar(
                            out=hsb[0][:, h4, :], in0=hp[(r, h4)][:, :],
                            scalar1=b1_t[:, h4:h4 + 1], scalar2=0.0,
                            op0=ALU.add, op1=ALU.max)
                    else:
                        nc.scalar.activation(
                            out=hsb[1][:, h4 - 2, :], in_=hp[(r, h4)][:, :],
                            func=ACT.Relu, bias=b1_t[:, h4:h4 + 1], scale=1.0)
                for hi, (d, zd) in enumerate(((DF, zf_d), (DC, zc_d))):
                    c0 = 0 if hi == 0 else DF
                    zp = ps.tile([d, RB], F32, tag=f"hp{r}{hi}", name=f"zp{r}{hi}")
                    if layer2_fp8:
                        nc.tensor.matmul(
                            out=zp[:, :], lhsT=w2_t[:, :, c0:c0 + d],
                            rhs=hsb[hi][:, :, :],
                            start=True, stop=True, perf_mode=DR)
                    else:
                        for kk in range(2):
                            nc.tensor.matmul(
                                out=zp[:, :], lhsT=w2_t[:, kk, c0:c0 + d],
                                rhs=hsb[hi][:, kk, :],
                                start=(kk == 0), stop=(kk == 1))
                    zt = zb.tile([d, RB], BF16, tag=f"z{hi}")
                    if hi == 0:
                        nc.vector.tensor_scalar(
                            out=zt[:, :], in0=zp[:, :], scalar1=1.0,
                            scalar2=None, op0=ALU.mult)
                    else:
                        nc.scalar.activation(
                            out=zt[:, :], in_=zp[:, :], func=ACT.Copy,
                            bias=0.0, scale=1.0)
                    nc.sync.dma_start(out=zd[:, r * RB:(r + 1) * RB], in_=zt[:, :])
    nc.compile()
    return nc


# --------------------------------------------------------------------------
# Launch 2: SIM
# --------------------------------------------------------------------------
def _build_sim_nc(n_fg, n_valid, nblk, act_ext):
    A = nblk * 128
    pieces, order, NA, ND = _sim_piece_table(n_fg, n_valid, act_ext)
    act_scale = 1.0 / (ZSCALE * ZSCALE * TAU)
    sch_c, sch_b = _schraudolph_consts(act_scale)

    nc = bacc.Bacc(trn_type="TRN2", num_devices=N_CORES, debug=False)
    zfk_d = nc.dram_tensor("zfk", [32, 2, N], F8, kind="ExternalInput")
    zck_d = nc.dram_tensor("zck", [64, 2, N], F8, kind="ExternalInput")
    zfa_d = nc.dram_tensor("zfa", [32, 2, A], F8, kind="ExternalInput")
    zca_d = nc.dram_tensor("zca", [64, 2, A], F8, kind="ExternalInput")
    sta_d = nc.dram_tensor("sta", [nblk, 128, NA], F32, kind="ExternalOutput")
    std_d = nc.dram_tensor("std", [nblk, 128, ND], F32, kind="ExternalOutput")

    by_group = {}
    for p in pieces:
        by_group.setdefault((p["head"], p["g"]), []).append(p)

    H = N // 2  # keys per half-tile (separate tiles so group RAW deps are tight)
    with tile.TileContext(nc) as tc:
        with (
            tc.tile_pool(name="keys", bufs=1) as keys,
            tc.tile_pool(name="anch", bufs=1) as anch,
            tc.tile_pool(name="sch", bufs=2) as sch,
            tc.tile_pool(name="st", bufs=2) as st,
            tc.tile_pool(name="ps", bufs=2, space="PSUM") as ps,
        ):
            zfa_t = anch.tile([32, 2, A], F8, tag="zfa")
            nc.sync.dma_start(out=zfa_t[:, :, :], in_=zfa_d[:, :, :])
            wu = st.tile([1, 8], F32, tag="wu")
            nc.vector.memset(wu[:, :], 0.0)
            nc.scalar.activation(out=wu[:, :], in_=wu[:, :], func=ACT.Exp, scale=1.0)
            zfk_t, zck_t = [], []
            zfk_t.append(keys.tile([32, 2, H], F8, tag="zfk0", name="zfk0"))
            nc.sync.dma_start(out=zfk_t[0][:, :, :], in_=zfk_d[:, :, 0:H])
            zca_t = anch.tile([64, 2, A], F8, tag="zca")
            nc.sync.dma_start(out=zca_t[:, :, :], in_=zca_d[:, :, :])
            zck_t.append(keys.tile([64, 2, H], F8, tag="zck0", name="zck0"))
            nc.sync.dma_start(out=zck_t[0][:, :, :], in_=zck_d[:, :, 0:H])
            zfk_t.append(keys.tile([32, 2, H], F8, tag="zfk1", name="zfk1"))
            nc.sync.dma_start(out=zfk_t[1][:, :, :], in_=zfk_d[:, :, H:N])
            zck_t.append(keys.tile([64, 2, H], F8, tag="zck1", name="zck1"))
            nc.sync.dma_start(out=zck_t[1][:, :, :], in_=zck_d[:, :, H:N])

            for ab in range(nblk):
                stA = st.tile([128, NA], F32, tag="stA")
                stD = st.tile([128, ND], F32, tag="stD")
                dummy = st.tile([128, G], BF16, tag="dummy")
                for oi, (head, g) in enumerate(order):
                    region = ps.tile([128, G], F32, tag="pr", name=f"pr{ab}_{oi}")
                    lhsT = (zfa_t if head == "f" else zca_t)[:, :, ab * 128:(ab + 1) * 128]
                    kt = (zfk_t if head == "f" else zck_t)[(g * G) // H]
                    koff = g * G - ((g * G) // H) * H
                    gp = by_group[(head, g)]
                    kmax = max(p["c1"] for p in gp)
                    nmm = int(math.ceil(kmax / 512))

                    def mm(kk):
                        nc.tensor.matmul(
                            out=region[:, kk * 512:(kk + 1) * 512],
                            lhsT=lhsT,
                            rhs=kt[:, :, koff + kk * 512:koff + (kk + 1) * 512],
                            start=True, stop=True, perf_mode=DR)

                    for kk in range(min(2, nmm)):
                        mm(kk)
                    for p in gp:
                        if p["eng"] == "A":
                            assert p["c1"] <= 1024
                            nc.scalar.activation(
                                out=region[:, p["c0"]:p["c1"]],
                                in_=region[:, p["c0"]:p["c1"]],
                                func=ACT.Exp, scale=act_scale,
                                accum_out=stA[:, p["stat"]:p["stat"] + 1])
                    for kk in range(2, nmm):
                        mm(kk)
                    i16 = {}
                    for eng, engine in (("P", nc.gpsimd), ("D", nc.vector)):
                        sp = [p for p in gp if p["eng"] == eng]
                        if not sp:
                            continue
                        c0 = min(p["c0"] for p in sp)
                        c1 = max(p["c1"] for p in sp)
                        t16 = sch.tile([128, c1 - c0], I16, tag=f"i16{eng}{oi}")
                        i16[eng] = (t16, c0)
                        engine.tensor_scalar(
                            out=t16[:, :], in0=region[:, c0:c1],
                            scalar1=sch_c, scalar2=sch_b,
                            op0=ALU.mult, op1=ALU.add)
                    for p in gp:
                        if p["eng"] in "PD":
                            t16, c0 = i16[p["eng"]]
                            nc.vector.tensor_scalar(
                                out=dummy[:, p["c0"]:p["c1"]],
                                in0=t16[:, p["c0"] - c0:p["c1"] - c0].bitcast(BF16),
                                scalar1=1.0, scalar2=0.0,
                                op0=ALU.mult, op1=ALU.add,
                                accum_out=stD[:, p["stat"] - NA:p["stat"] - NA + 1])
                nc.sync.dma_start(out=sta_d[ab, :, :], in_=stA[:, :])
                nc.sync.dma_start(out=std_d[ab, :, :], in_=stD[:, :])
    nc.compile()
    return nc


def _run(nc, in_maps, out_names):
    import time as _time

    if os.environ.get("CC_BASS_SIM") == "1":
        from concourse import bass_interp

        ncores = int(os.environ.get("CC_BASS_SIM_CORES", str(N_CORES)))
        results = []
        for m in range(ncores):
            sim = bass_interp.CoreSim(nc, core_id=m)
            for k, v in in_maps[m].items():
                sim.tensor(k)[:] = v
            if nc.partition_id_tensor is not None:
                sim.tensor(nc.partition_id_tensor.name)[:] = np.array(
                    [[m]], dtype=np.uint32)
            sim.simulate()
            results.append(
                {name: np.array(sim.mem_tensor(name)) for name in out_names})
        while len(results) < N_CORES:
            results.append(results[-1])
        return results
    t0 = _time.monotonic()
    res = run_bass_kernel_spmd(nc, in_maps, core_ids=list(range(N_CORES)))
    LAST_TIMES.append(_time.monotonic() - t0)
    LAST_RESULTS.append(res)
    return res.results


def kernel(**inputs):
    global LAST_RESULTS, LAST_TIMES
    LAST_RESULTS = []
    LAST_TIMES = []

    roi = np.ascontiguousarray(np.asarray(inputs["roi_feats"], dtype=np.float32))
    labels = np.asarray(inputs["labels"]).astype(np.int64)
    ious = np.asarray(inputs["ious"], dtype=np.float32)
    w1f = np.asarray(inputs["w1f"], dtype=np.float64)
    b1f = np.asarray(inputs["b1f"], dtype=np.float64)
    w2f = np.asarray(inputs["w2f"], dtype=np.float64)
    b2f = np.asarray(inputs["b2f"], dtype=np.float64)
    w1c = np.asarray(inputs["w1c"], dtype=np.float64)
    b1c = np.asarray(inputs["b1c"], dtype=np.float64)
    w2c = np.asarray(inputs["w2c"], dtype=np.float64)
    b2c = np.asarray(inputs["b2c"], dtype=np.float64)
    assert roi.shape == (N, C)

    ign = labels == -1
    fg = (labels > 0) & ~ign
    bg = (labels == 0) & ~ign
    anc = fg & (ious > IOU_THRESHOLD)

    perm = np.concatenate(
        [np.where(anc)[0], np.where(fg & ~anc)[0], np.where(bg)[0], np.where(ign)[0]])
    n_A = int(anc.sum())
    n_fg = int(fg.sum())
    n_valid = n_fg + int(bg.sum())
    if n_A == 0:
        return np.zeros(2, dtype=np.float32)

    x_s = roi[perm]
    labels_s = labels[perm]
    ious_s = ious[perm].astype(np.float64)

    # ---------------- launch 1: MLP ----------------
    w1_all = np.concatenate([w1f, w1c], axis=0)
    b1_all = np.concatenate([b1f, b1c], axis=0) * W1SCALE
    w1_q = (w1_all * W1SCALE).astype(NP8)
    x_q = x_s.astype(NP8)

    h_probe = np.maximum(
        x_q[:256].astype(np.float32) @ w1_q.astype(np.float32).T
        + b1_all.astype(np.float32), 0)
    layer2_fp8 = bool(h_probe.max() < 200.0)

    mlp_key = ("mlp", layer2_fp8)
    if mlp_key not in _NC_CACHE:
        _NC_CACHE[mlp_key] = _build_mlp_nc(layer2_fp8)
    nc1 = _NC_CACHE[mlp_key]

    KC4 = C // 256
    R = N // N_CORES
    w1_dr = np.ascontiguousarray(
        w1_q.T.reshape(KC4, 2, 128, HID2).transpose(2, 0, 1, 3))
    w2_all = np.concatenate([w2f, w2c], axis=0)
    w2dt = NP8 if layer2_fp8 else NPBF
    w2_dr = np.ascontiguousarray(
        w2_all.T.reshape(2, 128, DF + DC).transpose(1, 0, 2)).astype(w2dt)
    b1_dr = np.ascontiguousarray(
        b1_all.reshape(HID2 // 128, 128).T).astype(np.float32)

    xT_q = np.ascontiguousarray(x_q.T)
    shared1 = {"w1": w1_dr, "w2": w2_dr, "b1": b1_dr}
    in_maps1 = []
    for m in range(N_CORES):
        xm = xT_q[:, m * R:(m + 1) * R]
        x_dr = np.ascontiguousarray(
            xm.reshape(KC4, 2, 128, R).transpose(2, 0, 1, 3))
        in_maps1.append({"x": x_dr, **shared1})
    res1 = _run(nc1, in_maps1, ["zf", "zc"])

    zfT_raw = np.concatenate([r["zf"].astype(np.float64) for r in res1], axis=1)
    zcT_raw = np.concatenate([r["zc"].astype(np.float64) for r in res1], axis=1)

    def _normalize(zT_raw, b2):
        z = zT_raw.T + b2[None, :] * W1SCALE
        nrm = np.sqrt(np.sum(z * z, axis=1, keepdims=True)) / W1SCALE
        return z / W1SCALE / np.maximum(nrm, EPS)

    zfn = _normalize(zfT_raw, b2f)
    zcn = _normalize(zcT_raw, b2c)

    zfq = (zfn * ZSCALE).astype(NP8)
    zcq = (zcn * ZSCALE).astype(NP8)
    zfq64 = zfq.astype(np.float64)
    zcq64 = zcq.astype(np.float64)

    # ---------------- launch 2: sims ----------------
    nblk = max(1, math.ceil(math.ceil(n_A / N_CORES) / 128))
    A_pc = nblk * 128
    # ACT extension into cls G1 to balance engines (rounded to 16)
    act_ext = 966 // 16 * 16
    sim_key = ("sim", n_fg, n_valid, nblk, act_ext)
    if sim_key not in _NC_CACHE:
        _NC_CACHE[sim_key] = _build_sim_nc(n_fg, n_valid, nblk, act_ext)
    nc2 = _NC_CACHE[sim_key]
    pieces, _, NA, ND = _sim_piece_table(n_fg, n_valid, act_ext)

    def _dr(zq_cols):
        d = zq_cols.shape[0]
        return np.ascontiguousarray(zq_cols.reshape(2, d // 2, -1).transpose(1, 0, 2))

    zfqT = np.ascontiguousarray(zfq.T)   # [DF, N]
    zcqT = np.ascontiguousarray(zcq.T)   # [DC, N]
    in_maps2 = []
    for m in range(N_CORES):
        lo = min(m * A_pc, n_A)
        hi = min((m + 1) * A_pc, n_A)
        # local key order: own anchor window first
        local = np.concatenate([
            np.arange(lo, hi),
            np.arange(0, lo),
            np.arange(hi, N),
        ])
        aidx = np.minimum(np.arange(m * A_pc, (m + 1) * A_pc), n_A - 1)
        in_maps2.append({
            "zfk": _dr(zfqT[:, local]),
            "zck": _dr(zcqT[:, local]),
            "zfa": _dr(zfqT[:, aidx]),
            "zca": _dr(zcqT[:, aidx]),
        })
    res2 = _run(nc2, in_maps2, ["sta", "std"])

    NSTAT = len(pieces)
    stats = np.stack(
        [np.concatenate([r["sta"].reshape(A_pc, NA),
                         r["std"].reshape(A_pc, ND)], axis=1) for r in res2],
        axis=0).astype(np.float64)        # [cores, A_pc, NSTAT]

    # ---------------- host: final losses in float64 ----------------
    act_scale = 1.0 / (ZSCALE * ZSCALE * TAU)
    # piece -> class membership (per-core local column space; class sections
    # are preserved by the local reordering, so boundaries are global)
    numer_cols = [p["stat"] for p in pieces
                  if p["head"] == "f" and p["g"] * G + p["c1"] <= n_fg]
    denom_cols = [p["stat"] for p in pieces if p["head"] == "f"]
    denc_cols = [p["stat"] for p in pieces if p["head"] == "c"]

    out_rows = np.empty((n_A, NSTAT), dtype=np.float64)
    for m in range(N_CORES):
        lo = m * A_pc
        hi = min((m + 1) * A_pc, n_A)
        if hi > lo:
            out_rows[lo:hi] = stats[m, : hi - lo]
    stats = out_rows                      # [n_A, NSTAT]

    w_a = ious_s[:n_A]
    sdot_f = np.einsum("nd,nd->n", zfq64[:n_A], zfq64[:n_A])
    sdot_c = np.einsum("nd,nd->n", zcq64[:n_A], zcq64[:n_A])
    selfexp_f = np.exp(sdot_f * act_scale)
    selfexp_c = np.exp(sdot_c * act_scale)

    numer = stats[:, numer_cols].sum(1) - selfexp_f
    denom = stats[:, denom_cols].sum(1) - selfexp_f
    denom_c = stats[:, denc_cols].sum(1) - selfexp_c

    if n_fg - 1 > 0:
        li = -np.log((numer + EPS) / (denom + EPS))
        loss_fg = np.sum(li * w_a) / (np.sum(w_a) + EPS)
    else:
        loss_fg = 0.0

    lab_valid = labels_s[:n_valid]
    cnt = np.bincount(lab_valid, minlength=21)
    S = np.zeros((21, DC), dtype=np.float64)
    np.add.at(S, lab_valid, zcn[:n_valid])
    c_a = labels_s[:n_A]
    n_pos = (cnt[c_a] - 1).astype(np.float64)
    denom_log = np.log(np.maximum(denom_c, 1e-300))
    zca64 = zcn[:n_A]
    sum_pos = (np.einsum("nd,nd->n", zca64, S[c_a])
               - np.einsum("nd,nd->n", zca64, zca64)) / TAU
    li_c = -(sum_pos - n_pos * denom_log) / np.maximum(n_pos, 1.0)
    valid_c = n_pos > 0
    num2 = np.sum(np.where(valid_c, li_c * w_a, 0.0))
    den2 = np.sum(np.where(valid_c, w_a, 0.0))
    loss_cls = num2 / (den2 + EPS12)

    return np.stack([loss_fg, loss_cls]).astype(np.float32)


# revision 25
# speedup vs baseline: 1.2566x; 1.2524x over previous
"""Trainium2 Bass kernel for nn_MultiHeadContrastive (two-head contrastive loss).

Strategy (8 NeuronCores, two SPMD launches, no collectives):

  Launch 1 (MLP): rows of roi_feats are sorted by group
  (anchor / fg-low-iou / bg / ignore) on the host and sharded contiguously,
  1024 rows per core.  Both layers run in fp8e4(e4m3) with DoubleRow perf
  mode (weights pre-scaled by 64 on the host so fp8 stays in its normal
  range; the scale cancels in the host-side normalization).  Raw
  (pre-normalization, pre-b2) embeddings return as bf16; the host adds b2
  and L2-normalizes in float64.

  Launch 2 (SIM): each core owns nblk*128 anchor rows (lhsT, fp8 x16) and
  all N keys (rhs, fp8-DR).  Keys are RE-ORDERED per core so the core's own
  anchors come first: every anchor's self-similarity column then lands in
  group 0, which is always evaluated by ACT (exact exp), so the host can
  subtract exact self terms.  Per anchor block, sim rows are built in PSUM
  2048 keys at a time (2 regions, double buffered).  exp(sim/TAU) plus the
  masked range sums are computed by THREE engines concurrently:
    - ACT: exact exp + accum_out on whole groups (plus one extension piece)
    - Pool: Schraudolph exp approximation (affine in the exponent, stored
      int16, re-read as bf16) on its column span of the remaining groups
    - DVE: same Schraudolph on its span, plus ALL range sums of the int16
      tiles via tensor_scalar+accum_out (4x DVE perf mode).
  Because rows are sorted, all masked sums are plain column-range sums.

  Host: subtracts exact self terms, computes the class-positive term of the
  SupCon loss from per-class sums of z, applies logs/weights in float64.
"""

import math
import os

import numpy as np
import ml_dtypes

import concourse.bacc as bacc
import concourse.mybir as mybir
import concourse.tile as tile
from concourse.bass_utils import run_bass_kernel_spmd

N_CORES = 8
N, C = 8192, 1024
HID, DF, DC = 256, 64, 128
HID2 = 2 * HID
TAU = 0.2
EPS = 1e-8
EPS12 = 1e-12
IOU_THRESHOLD = 0.5

F32 = mybir.dt.float32
BF16 = mybir.dt.bfloat16
F8 = mybir.dt.float8e4
I16 = mybir.dt.int16
ACT = mybir.ActivationFunctionType
AX = mybir.AxisListType
ALU = mybir.AluOpType
DR = mybir.MatmulPerfMode.DoubleRow

NP8 = ml_dtypes.float8_e4m3
NPBF = ml_dtypes.bfloat16

W1SCALE = 64.0
ZSCALE = 16.0
SCH_SIG = 0.05394  # zero-mean rel. error under float->int16 truncation

LAST_RESULTS = []
LAST_TIMES = []
_NC_CACHE = {}

G = 2048
# per-group column spans (bank-quantized): ACT [0,1024) on its own 2-bank
# PSUM tile, Pool [1024,1536) and DVE [1536,2048) on 1-bank tiles each.
A_SH = 1024
P_SH = 512


def _schraudolph_consts(psum_scale):
    c = 128.0 * psum_scale * np.log2(np.e)
    b = 128.0 * (127.0 - SCH_SIG)
    return float(c), float(b)


# --------------------------------------------------------------------------
# SIM piece table (shared between device builder and host reassembly)
# --------------------------------------------------------------------------
def _sim_piece_table(n_fg, n_valid, act_ext):
    """Returns (pieces, order).

    pieces: list of dicts: head ('f'|'c'), group g, span [c0,c1) local to the
    group, engine in {'A','P','D'}, plus 'splits': sorted class-boundary cuts
    within the span (absolute head-column space).  Each (engine-span x split
    interval) becomes one stat column, assigned in order.

    order: group emission order [(head, g), ...] interleaving ACT-owned and
    Pool/DVE-owned groups for engine overlap.
    """
    NGH = N // G                  # 4 groups per head
    pieces = []

    def add(head, g, c0, c1, eng, cuts=()):
        lo = g * G
        cs = sorted({c0 + lo, c1 + lo} | {c for c in cuts if c0 + lo < c < c1 + lo})
        for a, b in zip(cs[:-1], cs[1:]):
            pieces.append(dict(head=head, g=g, c0=a - lo, c1=b - lo, eng=eng))

    def grp(head, g, L, cuts=()):
        # every group gets all three engines: ACT / Pool / DVE column spans
        a = min(A_SH, L)
        p = min(P_SH, max(0, L - a))
        add(head, g, 0, a, "A", cuts)
        if p > 0:
            add(head, g, a, a + p, "P", cuts)
        if a + p < L:
            add(head, g, a + p, L, "D", cuts)

    lim = {"f": N, "c": n_valid}
    order = []
    for g in range(NGH):
        for head in ("f", "c"):
            L = min(G, lim[head] - g * G)
            if L <= 0:
                continue
            order.append((head, g))
            grp(head, g, L, cuts=(n_fg,) if head == "f" else ())
    # stat numbering: ACT pieces first (their own stat tile), then Pool/DVE
    na = 0
    for p in pieces:
        if p["eng"] == "A":
            p["stat"] = na
            na += 1
    nd = 0
    for p in pieces:
        if p["eng"] != "A":
            p["stat"] = na + nd
            nd += 1
    return pieces, order, na, nd


# --------------------------------------------------------------------------
# Launch 1: MLP (both heads, fp8 DoubleRow)
# --------------------------------------------------------------------------
def _build_mlp_nc(layer2_fp8):
    R = N // N_CORES
    RB = 512
    NR = R // RB
    KC4 = C // 256
    NH4 = HID2 // 128

    nc = bacc.Bacc(trn_type="TRN2", num_devices=N_CORES, debug=False)
    x_d = nc.dram_tensor("x", [128, KC4, 2, R], F8, kind="ExternalInput")
    w1_d = nc.dram_tensor("w1", [128, KC4, 2, HID2], F8, kind="ExternalInput")
    w2dt = F8 if layer2_fp8 else BF16
    w2_d = nc.dram_tensor("w2", [128, 2, DF + DC], w2dt, kind="ExternalInput")
    b1_d = nc.dram_tensor("b1", [128, NH4], F32, kind="ExternalInput")
    zf_d = nc.dram_tensor("zf", [DF, R], BF16, kind="ExternalOutput")
    zc_d = nc.dram_tensor("zc", [DC, R], BF16, kind="ExternalOutput")

    hdt = F8 if layer2_fp8 else BF16
    with tile.TileContext(nc) as tc:
        with (
            tc.tile_pool(name="cst", bufs=1) as cst,
            tc.tile_pool(name="hb", bufs=2) as hb,
            tc.tile_pool(name="zb", bufs=2) as zb,
            tc.tile_pool(name="ps", bufs=1, space="PSUM") as ps,
        ):
            wu = cst.tile([1, 8], F32, tag="wu")
            nc.vector.memset(wu[:, :], 0.0)
            nc.scalar.activation(out=wu[:, :], in_=wu[:, :], func=ACT.Relu, scale=1.0)

            # DMA order: w1(k01), x0(k01), w1(k23), x0(k23), x1, b1, w2
            # separate tiles per DMA so RAW deps stay piece-accurate
            w1_t = [cst.tile([128, 2, 2, HID2], F8, tag=f"w1{i}", name=f"w1{i}") for i in range(2)]
            x_t = {}
            for r in range(NR):
                for i in range(2):
                    x_t[(r, i)] = cst.tile([128, 2, 2, RB], F8, tag=f"x{r}{i}", name=f"x{r}{i}")
            nc.sync.dma_start(out=w1_t[0][:, :, :, :], in_=w1_d[:, 0:2, :, :])
            nc.sync.dma_start(out=x_t[(0, 0)][:, :, :, :], in_=x_d[:, 0:2, :, 0:RB])
            nc.sync.dma_start(out=w1_t[1][:, :, :, :], in_=w1_d[:, 2:4, :, :])
            nc.sync.dma_start(out=x_t[(0, 1)][:, :, :, :], in_=x_d[:, 2:4, :, 0:RB])
            nc.sync.dma_start(out=x_t[(1, 0)][:, :, :, :], in_=x_d[:, 0:2, :, RB:R])
            nc.sync.dma_start(out=x_t[(1, 1)][:, :, :, :], in_=x_d[:, 2:4, :, RB:R])
            b1_t = cst.tile([128, NH4], F32, tag="b1")
            nc.sync.dma_start(out=b1_t[:, :], in_=b1_d[:, :])
            w2_t = cst.tile([128, 2, DF + DC], w2dt, tag="w2")
            nc.sync.dma_start(out=w2_t[:, :, :], in_=w2_d[:, :, :])

            # 8 psum banks: hp[r][h4] for both rblocks; z reuses drained banks
            hp = {(r, h4): ps.tile([128, RB], F32, tag=f"hp{r}{h4}",
                                   name=f"hp{r}{h4}")
                  for r in range(NR) for h4 in range(NH4)}
            for r in range(NR):
                # engine-pure hidden tiles: DVE writes head f, ACT head c
                hsb = [hb.tile([128, 2, RB], hdt, tag=f"h{r}f", name=f"h{r}f"),
                       hb.tile([128, 2, RB], hdt, tag=f"h{r}c", name=f"h{r}c")]
                for h4 in range(NH4):
                    for k in range(KC4):
                        nc.tensor.matmul(
                            out=hp[(r, h4)][:, :],
                            lhsT=w1_t[k // 2][:, k % 2, :, h4 * 128:(h4 + 1) * 128],
                            rhs=x_t[(r, k // 2)][:, k % 2, :, :],
                            start=(k == 0),
                            stop=(k == KC4 - 1),
                            perf_mode=DR,
                        )
                    if h4 < 2:
                        nc.vector.tensor_scalar(
                            out=hsb[0][:, h4, :], in0=hp[(r, h4)][:, :],
                            scalar1=b1_t[:, h4:h4 + 1], scalar2=0.0,
                            op0=ALU.add, op1=ALU.max)
                    else:
                        nc.scalar.activation(
                            out=hsb[1][:, h4 - 2, :], in_=hp[(r, h4)][:, :],
                            func=ACT.Relu, bias=b1_t[:, h4:h4 + 1], scale=1.0)
                for hi, (d, zd) in enumerate(((DF, zf_d), (DC, zc_d))):
                    c0 = 0 if hi == 0 else DF
                    zp = ps.tile([d, RB], F32, tag=f"hp{r}{hi}", name=f"zp{r}{hi}")
                    if layer2_fp8:
                        nc.tensor.matmul(
                            out=zp[:, :], lhsT=w2_t[:, :, c0:c0 + d],
                            rhs=hsb[hi][:, :, :],
                            start=True, stop=True, perf_mode=DR)
                    else:
                        for kk in range(2):
                            nc.tensor.matmul(
                                out=zp[:, :], lhsT=w2_t[:, kk, c0:c0 + d],
                                rhs=hsb[hi][:, kk, :],
                                start=(kk == 0), stop=(kk == 1))
                    zt = zb.tile([d, RB], BF16, tag=f"z{hi}")
                    if hi == 0:
                        nc.vector.tensor_scalar(
                            out=zt[:, :], in0=zp[:, :], scalar1=1.0,
                            scalar2=None, op0=ALU.mult)
                    else:
                        nc.scalar.activation(
                            out=zt[:, :], in_=zp[:, :], func=ACT.Copy,
                            bias=0.0, scale=1.0)
                    nc.sync.dma_start(out=zd[:, r * RB:(r + 1) * RB], in_=zt[:, :])
    nc.compile()
    return nc


# --------------------------------------------------------------------------
# Launch 2: SIM
# --------------------------------------------------------------------------
def _build_sim_nc(n_fg, n_valid, nblk, act_ext):
    A = nblk * 128
    pieces, order, NA, ND = _sim_piece_table(n_fg, n_valid, act_ext)
    act_scale = 1.0 / (ZSCALE * ZSCALE * TAU)
    sch_c, sch_b = _schraudolph_consts(act_scale)

    nc = bacc.Bacc(trn_type="TRN2", num_devices=N_CORES, debug=False)
    zfk_d = nc.dram_tensor("zfk", [32, 2, N], F8, kind="ExternalInput")
    zck_d = nc.dram_tensor("zck", [64, 2, N], F8, kind="ExternalInput")
    zfa_d = nc.dram_tensor("zfa", [32, 2, A], F8, kind="ExternalInput")
    zca_d = nc.dram_tensor("zca", [64, 2, A], F8, kind="ExternalInput")
    sta_d = nc.dram_tensor("sta", [nblk, 128, NA], F32, kind="ExternalOutput")
    std_d = nc.dram_tensor("std", [nblk, 128, ND], F32, kind="ExternalOutput")

    by_group = {}
    for p in pieces:
        by_group.setdefault((p["head"], p["g"]), []).append(p)

    H = N // 2  # keys per half-tile (separate tiles so group RAW deps are tight)
    with tile.TileContext(nc) as tc:
        with (
            tc.tile_pool(name="keys", bufs=1) as keys,
            tc.tile_pool(name="anch", bufs=1) as anch,
            tc.tile_pool(name="sch", bufs=2) as sch,
            tc.tile_pool(name="st", bufs=2) as st,
            tc.tile_pool(name="ps", bufs=2, space="PSUM") as ps,
        ):
            zfa_t = anch.tile([32, 2, A], F8, tag="zfa")
            nc.sync.dma_start(out=zfa_t[:, :, :], in_=zfa_d[:, :, :])
            wu = st.tile([1, 8], F32, tag="wu")
            nc.vector.memset(wu[:, :], 0.0)
            nc.scalar.activation(out=wu[:, :], in_=wu[:, :], func=ACT.Exp, scale=1.0)
            zfk_t, zck_t = [], []
            zfk_t.append(keys.tile([32, 2, H], F8, tag="zfk0", name="zfk0"))
            nc.sync.dma_start(out=zfk_t[0][:, :, :], in_=zfk_d[:, :, 0:H])
            zca_t = anch.tile([64, 2, A], F8, tag="zca")
            nc.sync.dma_start(out=zca_t[:, :, :], in_=zca_d[:, :, :])
            zck_t.append(keys.tile([64, 2, H], F8, tag="zck0", name="zck0"))
            nc.sync.dma_start(out=zck_t[0][:, :, :], in_=zck_d[:, :, 0:H])
            zfk_t.append(keys.tile([32, 2, H], F8, tag="zfk1", name="zfk1"))
            nc.sync.dma_start(out=zfk_t[1][:, :, :], in_=zfk_d[:, :, H:N])
            zck_t.append(keys.tile([64, 2, H], F8, tag="zck1", name="zck1"))
            nc.sync.dma_start(out=zck_t[1][:, :, :], in_=zck_d[:, :, H:N])

            for ab in range(nblk):
                stA = st.tile([128, NA], F32, tag="stA")
                stD = st.tile([128, ND], F32, tag="stD")
                dummy = st.tile([128, G], BF16, tag="dummy")
                for oi, (head, g) in enumerate(order):
                    lhsT = (zfa_t if head == "f" else zca_t)[:, :, ab * 128:(ab + 1) * 128]
                    kt = (zfk_t if head == "f" else zck_t)[(g * G) // H]
                    koff = g * G - ((g * G) // H) * H
                    gp = by_group[(head, g)]
                    L = max(p["c1"] for p in gp)
                    # separate PSUM tiles per consumer engine so the three
                    # engines run concurrently (deps are tile-granular)
                    pA = ps.tile([128, A_SH], F32, tag="pA", name=f"pA{ab}_{oi}")
                    asp = min(A_SH, L)
                    psp = min(P_SH, max(0, L - A_SH))
                    dsp = max(0, L - A_SH - P_SH)
                    pP = pD = None
                    for kk in range(int(math.ceil(asp / 512))):
                        hi = min(asp, (kk + 1) * 512)
                        nc.tensor.matmul(
                            out=pA[:, kk * 512:hi], lhsT=lhsT,
                            rhs=kt[:, :, koff + kk * 512:koff + hi],
                            start=True, stop=True, perf_mode=DR)
                    for p in gp:
                        if p["eng"] == "A":
                            nc.scalar.activation(
                                out=pA[:, p["c0"]:p["c1"]],
                                in_=pA[:, p["c0"]:p["c1"]],
                                func=ACT.Exp, scale=act_scale,
                                accum_out=stA[:, p["stat"]:p["stat"] + 1])
                    if psp > 0:
                        pP = ps.tile([128, P_SH], F32, tag="pP", name=f"pP{ab}_{oi}")
                        nc.tensor.matmul(
                            out=pP[:, 0:psp], lhsT=lhsT,
                            rhs=kt[:, :, koff + A_SH:koff + A_SH + psp],
                            start=True, stop=True, perf_mode=DR)
                    if dsp > 0:
                        pD = ps.tile([128, P_SH], F32, tag="pD", name=f"pD{ab}_{oi}")
                        nc.tensor.matmul(
                            out=pD[:, 0:dsp], lhsT=lhsT,
                            rhs=kt[:, :, koff + A_SH + P_SH:koff + L],
                            start=True, stop=True, perf_mode=DR)
                    i16 = {}
                    for eng, engine, src, base in (("P", nc.gpsimd, pP, A_SH),
                                                   ("D", nc.vector, pD, A_SH + P_SH)):
                        sp = [p for p in gp if p["eng"] == eng]
                        if not sp:
                            continue
                        c0 = min(p["c0"] for p in sp)
                        c1 = max(p["c1"] for p in sp)
                        t16 = sch.tile([128, c1 - c0], I16, tag=f"i16{eng}{oi}",
                                       name=f"i16{eng}{ab}_{oi}")
                        i16[eng] = (t16, c0)
                        engine.tensor_scalar(
                            out=t16[:, :], in0=src[:, c0 - base:c1 - base],
                            scalar1=sch_c, scalar2=sch_b,
                            op0=ALU.mult, op1=ALU.add)
                    for p in gp:
                        if p["eng"] in "PD":
                            t16, c0 = i16[p["eng"]]
                            nc.vector.tensor_scalar(
                                out=dummy[:, p["c0"]:p["c1"]],
                                in0=t16[:, p["c0"] - c0:p["c1"] - c0].bitcast(BF16),
                                scalar1=1.0, scalar2=0.0,
                                op0=ALU.mult, op1=ALU.add,
                                accum_out=stD[:, p["stat"] - NA:p["stat"] - NA + 1])
                nc.sync.dma_start(out=sta_d[ab, :, :], in_=stA[:, :])
                nc.sync.dma_start(out=std_d[ab, :, :], in_=stD[:, :])
    nc.compile()
    return nc


def _run(nc, in_maps, out_names):
    import time as _time

    if os.environ.get("CC_BASS_SIM") == "1":
        from concourse import bass_interp

        ncores = int(os.environ.get("CC_BASS_SIM_CORES", str(N_CORES)))
        results = []
        for m in range(ncores):
            sim = bass_interp.CoreSim(nc, core_id=m)
            for k, v in in_maps[m].items():
                sim.tensor(k)[:] = v
            if nc.partition_id_tensor is not None:
                sim.tensor(nc.partition_id_tensor.name)[:] = np.array(
                    [[m]], dtype=np.uint32)
            sim.simulate()
            results.append(
                {name: np.array(sim.mem_tensor(name)) for name in out_names})
        while len(results) < N_CORES:
            results.append(results[-1])
        return results
    t0 = _time.monotonic()
    res = run_bass_kernel_spmd(nc, in_maps, core_ids=list(range(N_CORES)))
    LAST_TIMES.append(_time.monotonic() - t0)
    LAST_RESULTS.append(res)
    return res.results


def kernel(**inputs):
    global LAST_RESULTS, LAST_TIMES
    LAST_RESULTS = []
    LAST_TIMES = []

    roi = np.ascontiguousarray(np.asarray(inputs["roi_feats"], dtype=np.float32))
    labels = np.asarray(inputs["labels"]).astype(np.int64)
    ious = np.asarray(inputs["ious"], dtype=np.float32)
    w1f = np.asarray(inputs["w1f"], dtype=np.float64)
    b1f = np.asarray(inputs["b1f"], dtype=np.float64)
    w2f = np.asarray(inputs["w2f"], dtype=np.float64)
    b2f = np.asarray(inputs["b2f"], dtype=np.float64)
    w1c = np.asarray(inputs["w1c"], dtype=np.float64)
    b1c = np.asarray(inputs["b1c"], dtype=np.float64)
    w2c = np.asarray(inputs["w2c"], dtype=np.float64)
    b2c = np.asarray(inputs["b2c"], dtype=np.float64)
    assert roi.shape == (N, C)

    ign = labels == -1
    fg = (labels > 0) & ~ign
    bg = (labels == 0) & ~ign
    anc = fg & (ious > IOU_THRESHOLD)

    perm = np.concatenate(
        [np.where(anc)[0], np.where(fg & ~anc)[0], np.where(bg)[0], np.where(ign)[0]])
    n_A = int(anc.sum())
    n_fg = int(fg.sum())
    n_valid = n_fg + int(bg.sum())
    if n_A == 0:
        return np.zeros(2, dtype=np.float32)

    x_s = roi[perm]
    labels_s = labels[perm]
    ious_s = ious[perm].astype(np.float64)

    # ---------------- launch 1: MLP ----------------
    w1_all = np.concatenate([w1f, w1c], axis=0)
    b1_all = np.concatenate([b1f, b1c], axis=0) * W1SCALE
    w1_q = (w1_all * W1SCALE).astype(NP8)
    x_q = x_s.astype(NP8)

    h_probe = np.maximum(
        x_q[:256].astype(np.float32) @ w1_q.astype(np.float32).T
        + b1_all.astype(np.float32), 0)
    layer2_fp8 = bool(h_probe.max() < 200.0)

    mlp_key = ("mlp", layer2_fp8)
    if mlp_key not in _NC_CACHE:
        _NC_CACHE[mlp_key] = _build_mlp_nc(layer2_fp8)
    nc1 = _NC_CACHE[mlp_key]

    KC4 = C // 256
    R = N // N_CORES
    w1_dr = np.ascontiguousarray(
        w1_q.T.reshape(KC4, 2, 128, HID2).transpose(2, 0, 1, 3))
    w2_all = np.concatenate([w2f, w2c], axis=0)
    w2dt = NP8 if layer2_fp8 else NPBF
    w2_dr = np.ascontiguousarray(
        w2_all.T.reshape(2, 128, DF + DC).transpose(1, 0, 2)).astype(w2dt)
    b1_dr = np.ascontiguousarray(
        b1_all.reshape(HID2 // 128, 128).T).astype(np.float32)

    xT_q = np.ascontiguousarray(x_q.T)
    shared1 = {"w1": w1_dr, "w2": w2_dr, "b1": b1_dr}
    in_maps1 = []
    for m in range(N_CORES):
        xm = xT_q[:, m * R:(m + 1) * R]
        x_dr = np.ascontiguousarray(
            xm.reshape(KC4, 2, 128, R).transpose(2, 0, 1, 3))
        in_maps1.append({"x": x_dr, **shared1})
    res1 = _run(nc1, in_maps1, ["zf", "zc"])

    zfT_raw = np.concatenate([r["zf"].astype(np.float64) for r in res1], axis=1)
    zcT_raw = np.concatenate([r["zc"].astype(np.float64) for r in res1], axis=1)

    def _normalize(zT_raw, b2):
        z = zT_raw.T + b2[None, :] * W1SCALE
        nrm = np.sqrt(np.sum(z * z, axis=1, keepdims=True)) / W1SCALE
        return z / W1SCALE / np.maximum(nrm, EPS)

    zfn = _normalize(zfT_raw, b2f)
    zcn = _normalize(zcT_raw, b2c)

    zfq = (zfn * ZSCALE).astype(NP8)
    zcq = (zcn * ZSCALE).astype(NP8)
    zfq64 = zfq.astype(np.float64)
    zcq64 = zcq.astype(np.float64)

    # ---------------- launch 2: sims ----------------
    nblk = max(1, math.ceil(math.ceil(n_A / N_CORES) / 128))
    A_pc = nblk * 128
    # ACT extension into cls G1 to balance engines (rounded to 16)
    act_ext = 966 // 16 * 16
    sim_key = ("sim", n_fg, n_valid, nblk, act_ext)
    if sim_key not in _NC_CACHE:
        _NC_CACHE[sim_key] = _build_sim_nc(n_fg, n_valid, nblk, act_ext)
    nc2 = _NC_CACHE[sim_key]
    pieces, _, NA, ND = _sim_piece_table(n_fg, n_valid, act_ext)

    def _dr(zq_cols):
        d = zq_cols.shape[0]
        return np.ascontiguousarray(zq_cols.reshape(2, d // 2, -1).transpose(1, 0, 2))

    zfqT = np.ascontiguousarray(zfq.T)   # [DF, N]
    zcqT = np.ascontiguousarray(zcq.T)   # [DC, N]
    in_maps2 = []
    for m in range(N_CORES):
        lo = min(m * A_pc, n_A)
        hi = min((m + 1) * A_pc, n_A)
        # local key order: own anchor window first
        local = np.concatenate([
            np.arange(lo, hi),
            np.arange(0, lo),
            np.arange(hi, N),
        ])
        aidx = np.minimum(np.arange(m * A_pc, (m + 1) * A_pc), n_A - 1)
        in_maps2.append({
            "zfk": _dr(zfqT[:, local]),
            "zck": _dr(zcqT[:, local]),
            "zfa": _dr(zfqT[:, aidx]),
            "zca": _dr(zcqT[:, aidx]),
        })
    res2 = _run(nc2, in_maps2, ["sta", "std"])

    NSTAT = len(pieces)
    stats = np.stack(
        [np.concatenate([r["sta"].reshape(A_pc, NA),
                         r["std"].reshape(A_pc, ND)], axis=1) for r in res2],
        axis=0).astype(np.float64)        # [cores, A_pc, NSTAT]

    # ---------------- host: final losses in float64 ----------------
    act_scale = 1.0 / (ZSCALE * ZSCALE * TAU)
    # piece -> class membership (per-core local column space; class sections
    # are preserved by the local reordering, so boundaries are global)
    numer_cols = [p["stat"] for p in pieces
                  if p["head"] == "f" and p["g"] * G + p["c1"] <= n_fg]
    denom_cols = [p["stat"] for p in pieces if p["head"] == "f"]
    denc_cols = [p["stat"] for p in pieces if p["head"] == "c"]

    out_rows = np.empty((n_A, NSTAT), dtype=np.float64)
    for m in range(N_CORES):
        lo = m * A_pc
        hi = min((m + 1) * A_pc, n_A)
        if hi > lo:
            out_rows[lo:hi] = stats[m, : hi - lo]
    stats = out_rows                      # [n_A, NSTAT]

    w_a = ious_s[:n_A]
    sdot_f = np.einsum("nd,nd->n", zfq64[:n_A], zfq64[:n_A])
    sdot_c = np.einsum("nd,nd->n", zcq64[:n_A], zcq64[:n_A])
    selfexp_f = np.exp(sdot_f * act_scale)
    selfexp_c = np.exp(sdot_c * act_scale)

    numer = stats[:, numer_cols].sum(1) - selfexp_f
    denom = stats[:, denom_cols].sum(1) - selfexp_f
    denom_c = stats[:, denc_cols].sum(1) - selfexp_c

    if n_fg - 1 > 0:
        li = -np.log((numer + EPS) / (denom + EPS))
        loss_fg = np.sum(li * w_a) / (np.sum(w_a) + EPS)
    else:
        loss_fg = 0.0

    lab_valid = labels_s[:n_valid]
    cnt = np.bincount(lab_valid, minlength=21)
    S = np.zeros((21, DC), dtype=np.float64)
    np.add.at(S, lab_valid, zcn[:n_valid])
    c_a = labels_s[:n_A]
    n_pos = (cnt[c_a] - 1).astype(np.float64)
    denom_log = np.log(np.maximum(denom_c, 1e-300))
    zca64 = zcn[:n_A]
    sum_pos = (np.einsum("nd,nd->n", zca64, S[c_a])
               - np.einsum("nd,nd->n", zca64, zca64)) / TAU
    li_c = -(sum_pos - n_pos * denom_log) / np.maximum(n_pos, 1.0)
    valid_c = n_pos > 0
    num2 = np.sum(np.where(valid_c, li_c * w_a, 0.0))
    den2 = np.sum(np.where(valid_c, w_a, 0.0))
    loss_cls = num2 / (den2 + EPS12)

    return np.stack([loss_fg, loss_cls]).astype(np.float32)
